# revision 36
# baseline (speedup 1.0000x reference)
import sys

sys.path.insert(0, "/opt/trn_rl_repo")

import numpy as np

import concourse.bass as bass
import concourse.tile as tile
from concourse import bacc, mybir
from concourse._compat import get_trn_type

EPS = 1e-6

BS, NSEQ, NB, NC_, ML = 32, 24, 196, 196, 6
BPC = 4            # batches per core
NCORES = 8
P = 112            # partition chunk for (b,i) rows: 4*196=784 = 7*112
NCHUNK = 7
EBLK = 8           # e-rows per scatter block: f = 8*196 = 1568
NEB = 3            # 24 = 3*8
FB = EBLK * NB     # 1568
EM = NSEQ * NB     # 4704
HALF = 98          # m-half for C^T chunks: 196 = 2*98
NKT = NSEQ * 2     # 48 C^T chunks (e, half)
ROWS = BPC * NB    # 784
INVALID = 255      # idx_bnd u8 invalid marker; mapped negative on device

# ---- packed single-input layout (per core), byte offsets ----
# f32 section first so every offset stays 4-aligned, then bf16, then 1-byte.
_PACK_SPEC = [
    ("eps4",  "f32",  (BPC, ML)),
    ("kcls4", "f32",  (BPC, NB)),
    ("wr",    "bf16", (BPC, ML * NB)),
    ("ea0",   "bf16", (BPC * NSEQ, NB)),
    ("Mt",    "u8",   (128, ML * NSEQ)),
    ("sel1",  "u8",   (128, ML * BPC)),
    ("sel2",  "u8",   (BPC, ML * 128)),
    ("spo",   "f8",   (ROWS, NSEQ, NC_)),
    ("roi",   "u8",   (ROWS, NC_)),
    ("ctx",   "u8",   (NCHUNK, P, NC_)),
]
_DTSIZE = {"f32": 4, "bf16": 2, "f8": 1, "u8": 1}
PACK_OFF = {}
_off = 0
for _n, _dt, _shape in _PACK_SPEC:
    PACK_OFF[_n] = _off
    _sz = _DTSIZE[_dt]
    for _d in _shape:
        _sz *= _d
    _off += _sz
PACK_BYTES = _off


def _host_prep(trav, adj, ent, spo, ctx, roi_cls, roi_mask, w_child, out=None):
    """Per-core (4-batch slice) host index/mask prep. Only int-derived
    index/mask/selector tensors and input reshapes/dtype casts — no float
    math on the attention data. Returns one packed u8 blob (written into
    `out` if given, so per-core packs can fill a global buffer in place)."""
    import ml_dtypes
    f32, u8, bf16 = np.float32, np.uint8, ml_dtypes.bfloat16
    f8 = ml_dtypes.float8_e4m3
    kcls = (roi_cls != -1).astype(f32)                     # [4, 196]

    # raw per-row ctx indices; the sort rank, segment boundaries, and
    # segment-continue flags are all derived on device (counting sort)
    ctx_rows = ctx.reshape(ROWS, NC_)                      # [784, 196]

    def chunks(a):  # [784, F] -> [7, 112, F]
        return np.ascontiguousarray(a.reshape(NCHUNK, P, -1))

    Mt = np.zeros((128, ML * NSEQ), dtype=u8)
    sel1 = np.zeros((128, ML * BPC), dtype=u8)
    sel2 = np.zeros((BPC, ML * 128), dtype=u8)
    w_rows = np.zeros((BPC, ML * NB), dtype=bf16)
    eps4 = np.zeros((BPC, ML), dtype=f32)
    for t in range(ML):
        for b in range(BPC):
            p_raw = int(trav[b, t])
            p = max(p_raw, 0)
            edges = adj[b, p]
            cm = (edges >= 0) & (p_raw >= 0)
            ec = np.maximum(edges, 0)
            nch = int(cm.sum())
            for j in range(NSEQ):
                if cm[j]:
                    Mt[b * 32 + j, t * NSEQ + int(ec[j])] = 1
            sel1[b * 32 + p, t * BPC + b] = 1
            if nch > 0 and p_raw >= 0:
                sel2[b, t * 128 + b * 32 + p] = 1
            w_rows[b, t * NB:(t + 1) * NB] = w_child[b, p].astype(bf16)
            eps4[b, t] = max(nch, 1) * EPS

    ea0 = np.ascontiguousarray(ent.reshape(BPC * NSEQ, NB).astype(bf16))
    # fold the per-row k_cls mask into roi (both binary): w3 = roi^3*kcls
    roi_eff = roi_mask.astype(u8).reshape(ROWS, NC_) \
        & (roi_cls != -1).reshape(ROWS, 1).astype(u8)

    sections = {
        "sel1": sel1, "sel2": sel2, "eps4": eps4,
        "ea0": ea0, "kcls4": kcls.astype(f32),
        "Mt": Mt, "wr": w_rows,
        "spo": np.ascontiguousarray(
            spo.transpose(0, 2, 1, 3)).astype(f8).reshape(ROWS, NSEQ, NC_),
        "roi": roi_eff,
        "ctx": chunks(ctx_rows.astype(u8)),
    }
    pack = out if out is not None else np.empty(PACK_BYTES, np.uint8)
    assert pack.nbytes == PACK_BYTES
    off = 0
    for n, _, _ in _PACK_SPEC:
        b = np.ascontiguousarray(sections[n]).reshape(-1).view(np.uint8)
        pack[off:off + b.nbytes] = b
        off += b.nbytes
    assert off == PACK_BYTES
    return {"pack": pack}


def _prep_all(traversal_lists, adj_matrices, ent_attn, spo_attn,
              ctx_idx_adjusted, roi_cls, roi_mask, weight_on_children):
    """Host prep for all 8 cores, packed directly into the global
    (NCORES*PACK_BYTES,) transfer buffer (shard k = core k's pack)."""
    gpack = np.empty(NCORES * PACK_BYTES, np.uint8)
    for k in range(NCORES):
        s = slice(k * BPC, (k + 1) * BPC)
        _host_prep(
            np.asarray(traversal_lists[s]), np.asarray(adj_matrices[s]),
            np.asarray(ent_attn[s]), np.asarray(spo_attn[s]),
            np.asarray(ctx_idx_adjusted[s]), np.asarray(roi_cls[s]),
            np.asarray(roi_mask[s]), np.asarray(weight_on_children[s]),
            out=gpack[k * PACK_BYTES:(k + 1) * PACK_BYTES],
        )
    return {"pack": gpack}


def build_bass():
    f32 = mybir.dt.float32
    bf16 = mybir.dt.bfloat16
    i16 = mybir.dt.int16
    f8 = mybir.dt.float8e4
    u8 = mybir.dt.uint8
    DT = {"f32": f32, "bf16": bf16, "f8": f8, "u8": u8}
    nc = bacc.Bacc(get_trn_type() or "TRN2", target_bir_lowering=False)

    pack_d = nc.dram_tensor("pack", (PACK_BYTES,), u8, kind="ExternalInput")
    out_d = nc.dram_tensor("ea_out", (BPC * NSEQ, NB), bf16, kind="ExternalOutput")

    spec = {n: (dt, shape) for n, dt, shape in _PACK_SPEC}

    def view(name, sub_off=0, shape=None):
        dt, full_shape = spec[name]
        shape = shape if shape is not None else full_shape
        n = 1
        for d in shape:
            n *= d
        sz = _DTSIZE[dt]
        ap = pack_d[PACK_OFF[name] + sub_off * sz:
                    PACK_OFF[name] + (sub_off + n) * sz].bitcast(DT[dt])
        if len(shape) == 2:
            ap = ap.rearrange("(a b) -> a b", a=shape[0])
        elif len(shape) == 3:
            ap = ap.rearrange("(a b c) -> a b c", a=shape[0], b=shape[1])
        return ap

    with tile.TileContext(nc) as tc:
        with (
            tc.tile_pool(name="persist", bufs=1) as pp,
            tc.tile_pool(name="stage", bufs=2) as sp,
            tc.tile_pool(name="work", bufs=2) as wp,
            tc.tile_pool(name="small", bufs=2) as mp,
            tc.tile_pool(name="psA", bufs=2, space="PSUM") as psA,
            tc.tile_pool(name="psB", bufs=1, space="PSUM") as psB,
        ):
            # ---- persistent tiles ----
            CT = pp.tile([HALF, NKT * ROWS], bf16, tag="CT")
            ea = pp.tile([128, NB], f32, tag="ea")
            eam = pp.tile([128, NB], bf16, tag="eam")
            Mt = pp.tile([128, ML * NSEQ], bf16, tag="Mt")
            sel1 = pp.tile([128, ML * BPC], f32, tag="sel1")
            sel2 = pp.tile([BPC, ML * 128], f32, tag="sel2")
            wr = pp.tile([BPC, ML * NB], bf16, tag="wr")
            eps4 = pp.tile([BPC, ML], f32, tag="eps4")
            kcls4 = pp.tile([BPC, NB], f32, tag="kcls4")
            mm1 = pp.tile([BPC, NB], f32, tag="mm1")
            ident = pp.tile([P, P], bf16, tag="ident")
            ones4 = pp.tile([HALF, BPC], f32, tag="ones4")
            acc = pp.tile([HALF, ROWS], f32, tag="acc")

            nc.sync.dma_start(wr[:], view("wr"))
            nc.sync.dma_start(eps4[:], view("eps4"))
            nc.sync.dma_start(kcls4[:], view("kcls4"))
            # 0/1 selector matrices ship as u8; convert on device
            for name, dst, shape in [("Mt", Mt, (128, ML * NSEQ)),
                                     ("sel1", sel1, (128, ML * BPC)),
                                     ("sel2", sel2, (BPC, ML * 128))]:
                stg = sp.tile(list(shape), u8, tag=f"{name}_u8")
                nc.sync.dma_start(stg[:], view(name))
                nc.scalar.copy(dst[:], stg[:])
            # identity for PE transposes: (iota_f - p) == 0
            idm = sp.tile([P, P], i16, tag="idm")
            nc.gpsimd.iota(idm[:], pattern=[[1, P]], base=0,
                           channel_multiplier=-1)
            nc.vector.tensor_scalar(ident[:], idm[:], 0, 1.0,
                                    op0=mybir.AluOpType.is_equal,
                                    op1=mybir.AluOpType.mult)
            # ea: bf16 on the wire, f32 accumulator tile on device
            ea_stg = sp.tile([128, NB], bf16, tag="ea_stg")
            nc.vector.memset(ea_stg[:], 0.0)
            for b in range(BPC):
                nc.sync.dma_start(
                    ea_stg[b * 32:b * 32 + NSEQ, :],
                    view("ea0", sub_off=b * NSEQ * NB, shape=(NSEQ, NB)))
            nc.vector.tensor_copy(ea[:], ea_stg[:])
            # kclsr rows via partition-broadcast DMA of kcls4
            klr = sp.tile([128, NB], f32, tag="klr")
            nc.vector.memset(klr[:], 0.0)
            for b in range(BPC):
                nc.sync.dma_start(
                    klr[b * 32:b * 32 + NSEQ, :],
                    view("kcls4", sub_off=b * NB,
                         shape=(1, NB)).broadcast_to((NSEQ, NB)))
            nc.vector.tensor_mul(eam[:], ea[:], klr[:])
            nc.vector.tensor_scalar_add(mm1[:], kcls4[:], -1.0)
            nc.vector.memset(ones4[:], 1.0)
            # per-row position iota 0..195 (f32), for unique sort keys
            ioti = sp.tile([P, NC_], i16, tag="ioti")
            nc.gpsimd.iota(ioti[:], pattern=[[1, NC_]], base=0,
                           channel_multiplier=0)
            iotf = pp.tile([P, NC_], f32, tag="iotf")
            nc.scalar.copy(iotf[:], ioti[:])

            # ---- per chunk: spo3 -> scatter -> scan -> extract -> transpose ----
            for c in range(NCHUNK):
                st = sp.tile([P, NSEQ, NC_], f8, tag="spost")
                nc.sync.dma_start(
                    st[:], view("spo", sub_off=c * P * NSEQ * NC_,
                                shape=(P, NSEQ, NC_)))
                rt = sp.tile([P, NC_], u8, tag="roist")
                nc.sync.dma_start(
                    rt[:], view("roi", sub_off=c * P * NC_, shape=(P, NC_)))
                # roi already has k_cls folded in on host (both binary masks)
                w3c = wp.tile([P, NC_], f32, tag="w3c")
                nc.scalar.copy(w3c[:], rt[:])
                sp3c = wp.tile([P, EM], bf16, tag="sp3c")
                w3b = w3c[:].unsqueeze(1).broadcast_to((P, NSEQ, NC_))
                nc.vector.tensor_mul(sp3c[:].rearrange("p (e c) -> p e c", e=NSEQ),
                                     st[:], w3b)
                # on-device counting sort of ctx: rank, sorted values,
                # segment boundaries, and segment-continue flags
                ctxc = wp.tile([P, NC_], u8, tag="ctxc")
                nc.sync.dma_start(
                    ctxc[:], view("ctx", sub_off=c * P * NC_, shape=(P, NC_)))
                key = wp.tile([P, NC_], f32, tag="key")
                nc.vector.tensor_scalar_mul(key[:], ctxc[:], 256.0)
                nc.vector.tensor_add(key[:], key[:], iotf[:])
                rank_f = wp.tile([P, NC_], f32, tag="rank_f")
                JBLK = 28
                scr = wp.tile([P, JBLK, NC_], bf16, tag="scr", bufs=1)
                for j0 in range(0, NC_, JBLK):
                    nc.vector.tensor_tensor(
                        scr[:],
                        key[:].unsqueeze(1).broadcast_to((P, JBLK, NC_)),
                        key[:, j0:j0 + JBLK].unsqueeze(2)
                            .broadcast_to((P, JBLK, NC_)),
                        op=mybir.AluOpType.is_lt)
                    nc.vector.tensor_reduce(
                        rank_f[:, j0:j0 + JBLK].unsqueeze(2), scr[:],
                        axis=mybir.AxisListType.X, op=mybir.AluOpType.add)
                rankc = wp.tile([P, NC_], i16, tag="rankc")
                nc.scalar.copy(rankc[:], rank_f[:])
                ctxb = wp.tile([P, NC_], bf16, tag="ctxb")
                nc.scalar.copy(ctxb[:], ctxc[:])
                msrt = wp.tile([P, NC_], bf16, tag="msrt")
                nc.gpsimd.local_scatter(msrt[:], ctxb[:], rankc[:],
                                        channels=P, num_elems=NC_,
                                        num_idxs=NC_)
                eqn = wp.tile([P, NC_], bf16, tag="eqn")
                nc.vector.memset(eqn[:], 0.0)
                nc.vector.tensor_tensor(eqn[:, :NC_ - 1], msrt[:, :NC_ - 1],
                                        msrt[:, 1:],
                                        op=mybir.AluOpType.is_equal)
                seg0 = wp.tile([P, NC_], bf16, tag="seg0")
                nc.vector.memset(seg0[:, 0:1], 0.0)
                nc.scalar.copy(seg0[:, 1:], eqn[:, :NC_ - 1])
                # bnd = m_sorted at segment ends, deeply negative elsewhere
                bb = wp.tile([P, NC_], bf16, tag="bb")
                nc.vector.tensor_scalar_mul(bb[:], eqn[:], -9816.0)
                nc.vector.tensor_add(bb[:], bb[:], msrt[:])
                bfix = wp.tile([P, NC_], i16, tag="bfix")
                nc.scalar.copy(bfix[:], bb[:])
                sigc = wp.tile([P, FB], i16, tag="sigc")
                bndc = wp.tile([P, FB], i16, tag="bndc")
                segc = wp.tile([P, FB], bf16, tag="segc")
                for e in range(EBLK):
                    s = slice(e * NC_, (e + 1) * NC_)
                    nc.vector.tensor_scalar_add(sigc[:, s], rankc[:], e * NC_)
                    nc.vector.tensor_scalar_add(bndc[:, s], bfix[:], e * NC_)
                    nc.scalar.copy(segc[:, s], seg0[:])
                Cmc = wp.tile([P, EM], bf16, tag="Cmc")
                for e in range(NEB):
                    fb0 = e * FB
                    srt = wp.tile([P, FB], bf16, tag="sorted")
                    nc.gpsimd.local_scatter(
                        srt[:], sp3c[:, fb0:fb0 + FB], sigc[:],
                        channels=P, num_elems=FB, num_idxs=FB,
                    )
                    scn = wp.tile([P, FB], bf16, tag="scan")
                    nc.vector.tensor_tensor_scan(
                        scn[:], segc[:], srt[:], 0.0,
                        op0=mybir.AluOpType.mult, op1=mybir.AluOpType.add,
                    )
                    nc.gpsimd.local_scatter(
                        Cmc[:, fb0:fb0 + FB], scn[:], bndc[:],
                        channels=P, num_elems=FB, num_idxs=FB,
                    )
                for g in range(NKT // 4):
                    pt4 = psA.tile([HALF, 4, P], bf16, tag="tp")
                    for j in range(4):
                        s = g * 4 + j
                        nc.tensor.transpose(
                            pt4[:, j, :], Cmc[:, s * HALF:(s + 1) * HALF],
                            ident[:])
                    dst = (CT[:, 4 * g * ROWS: 4 * (g + 1) * ROWS]
                           .rearrange("p (s r) -> p s r", s=4)
                           [:, :, c * P:(c + 1) * P])
                    nc.scalar.copy(dst, pt4[:])

            # ---- 6 sequential steps ----
            for t in range(ML):
                a4 = [mp.tile([HALF, NSEQ, BPC], bf16, tag=f"a4_{h}",
                              name=f"a4_{h}") for h in range(2)]
                for h in range(2):
                    for b in range(BPC):
                        aps = psA.tile([HALF, NSEQ], f32, tag="aps")
                        nc.tensor.matmul(
                            aps[:],
                            eam[b * 32:b * 32 + NSEQ, h * HALF:(h + 1) * HALF],
                            Mt[b * 32:b * 32 + NSEQ, t * NSEQ:(t + 1) * NSEQ],
                            start=True, stop=True,
                            tile_position=(b * 32, 0),
                        )
                        nc.scalar.copy(a4[h][:, :, b], aps[:])
                KPE = 34
                rps = [psB.tile([BPC, 2 * NB], f32, tag=f"rps{nb}",
                                name=f"rps{nb}") for nb in range(2)]
                for k in range(NKT):
                    e, h = k // 2, k % 2
                    if k < KPE:
                        for nb in range(2):
                            nc.tensor.matmul(
                                rps[nb][:],
                                a4[h][:, e, :],
                                CT[:, k * ROWS + nb * 2 * NB: k * ROWS + (nb + 1) * 2 * NB],
                                start=(k == 0), stop=False,
                            )
                    else:
                        for b in range(BPC):
                            nc.vector.scalar_tensor_tensor(
                                acc[:, b * NB:(b + 1) * NB],
                                CT[:, k * ROWS + b * NB: k * ROWS + (b + 1) * NB],
                                a4[h][:, e, b:b + 1],
                                acc[:, b * NB:(b + 1) * NB],
                                op0=mybir.AluOpType.mult,
                                op1=(mybir.AluOpType.add if k > KPE
                                     else mybir.AluOpType.bypass),
                            )
                for nb in range(2):
                    nc.tensor.matmul(
                        rps[nb][:], ones4[:],
                        acc[:, nb * 2 * NB:(nb + 1) * 2 * NB],
                        start=False, stop=(nb == 1),
                    )
                r4 = mp.tile([BPC, NB], f32, tag="r4")
                for nb in range(2):
                    rsb = mp.tile([BPC, 2 * NB], f32, tag=f"rsb{nb}",
                                  name=f"rsb{nb}", bufs=1)
                    nc.vector.tensor_copy(rsb[:], rps[nb][:])
                    for b in (2 * nb, 2 * nb + 1):
                        nc.sync.dma_start(
                            r4[b:b + 1, :],
                            rsb[b:b + 1, (b % 2) * NB:(b % 2) * NB + NB])
                nc.vector.tensor_scalar_add(r4[:], r4[:], eps4[:, t:t + 1])
                sps = psB.tile([BPC, NB], f32, tag="sps")
                nc.tensor.matmul(sps[:], sel1[:, t * BPC:(t + 1) * BPC], ea[:],
                                 start=True, stop=True)
                srow = mp.tile([BPC, NB], f32, tag="srow")
                nc.vector.tensor_copy(srow[:], sps[:])
                upd = mp.tile([BPC, NB], f32, tag="upd")
                nc.vector.tensor_mul(upd[:], r4[:], wr[:, t * NB:(t + 1) * NB])
                nc.vector.tensor_add(upd[:], upd[:], srow[:])
                nrm = mp.tile([BPC, 1], f32, tag="nrm")
                nc.vector.tensor_reduce(nrm[:], upd[:], axis=mybir.AxisListType.X,
                                        op=mybir.AluOpType.max,
                                        apply_absolute_value=True)
                nc.vector.tensor_scalar_max(nrm[:], nrm[:], 1.0)
                rec = mp.tile([BPC, 1], f32, tag="rec")
                nc.vector.reciprocal(rec[:], nrm[:])
                nc.vector.tensor_scalar_mul(upd[:], upd[:], rec[:])
                nc.vector.tensor_mul(upd[:], upd[:], kcls4[:])
                nc.vector.tensor_add(upd[:], upd[:], mm1[:])
                dd = mp.tile([BPC, 2 * NB], f32, tag="dd", bufs=1)
                nc.vector.tensor_sub(dd[:, :NB], upd[:], srow[:])
                nc.vector.tensor_mul(dd[:, NB:], dd[:, :NB], kcls4[:])
                wps = psB.tile([128, 2 * NB], f32, tag="wps")
                nc.tensor.matmul(wps[:], sel2[:, t * 128:(t + 1) * 128], dd[:],
                                 start=True, stop=True)
                nc.vector.tensor_add(ea[:], ea[:], wps[:, :NB])
                nc.vector.tensor_add(eam[:], eam[:], wps[:, NB:])

            eaout = pp.tile([128, NB], bf16, tag="eaout")
            nc.vector.tensor_copy(eaout[:], ea[:])
            for b in range(BPC):
                nc.sync.dma_start(out_d[b * NSEQ:(b + 1) * NSEQ, :],
                                  eaout[b * 32:b * 32 + NSEQ, :])

    nc.compile()
    return nc


_NC_CACHE = None
_RUN_CACHE = None


def _get_runner():
    """Build (once) a cached jitted dispatch for the compiled Bass module.

    Mirrors what bass_utils.run_bass_kernel_spmd does under axon
    (bass2jax.run_bass_via_pjrt), but keeps the jitted executable across
    calls so repeat dispatches skip re-trace/re-lowering.
    """
    global _NC_CACHE, _RUN_CACHE
    if _RUN_CACHE is not None:
        return _RUN_CACHE
    import jax
    from jax.sharding import Mesh, PartitionSpec
    from jax.experimental.shard_map import shard_map
    from concourse.bass2jax import (
        install_neuronx_cc_hook, _bass_exec_p, partition_id_tensor,
    )

    if _NC_CACHE is None:
        _NC_CACHE = build_bass()
    nc = _NC_CACHE
    install_neuronx_cc_hook()
    partition_name = nc.partition_id_tensor.name if nc.partition_id_tensor else None
    in_names, out_names, out_avals, zero_shapes = [], [], [], []
    for alloc in nc.m.functions[0].allocations:
        if not isinstance(alloc, mybir.MemoryLocationSet):
            continue
        name = alloc.memorylocations[0].name
        if alloc.kind == "ExternalInput":
            if name != partition_name:
                in_names.append(name)
        elif alloc.kind == "ExternalOutput":
            out_names.append(name)
            shape = tuple(alloc.tensor_shape)
            dtype = mybir.dt.np(alloc.dtype)
            out_avals.append(jax.core.ShapedArray(shape, dtype))
            zero_shapes.append((shape, dtype))
    n_params = len(in_names)
    n_outs = len(out_avals)
    all_names = list(in_names) + out_names
    if partition_name is not None:
        all_names.append(partition_name)
    donate = tuple(range(n_params, n_params + n_outs))

    def _body(*args):
        operands = list(args)
        if partition_name is not None:
            operands.append(partition_id_tensor())
        outs = _bass_exec_p.bind(
            *operands, out_avals=tuple(out_avals), in_names=tuple(all_names),
            out_names=tuple(out_names), lowering_input_output_aliases=(),
            sim_require_finite=True, sim_require_nnan=True, nc=nc)
        return tuple(outs)

    devices = jax.devices()[:NCORES]
    mesh = Mesh(np.asarray(devices), ("core",))
    sharded = jax.jit(
        shard_map(_body, mesh=mesh,
                  in_specs=(PartitionSpec("core"),) * (n_params + n_outs),
                  out_specs=(PartitionSpec("core"),) * n_outs,
                  check_rep=False),
        donate_argnums=donate, keep_unused=True)
    _RUN_CACHE = (sharded, in_names, out_names, out_avals, zero_shapes)
    return _RUN_CACHE


def _dispatch(global_map):
    """One full dispatch: H2D staging -> execute -> D2H fetch.

    `global_map` holds pre-sharded global arrays (axis 0 = core-major)."""
    sharded, in_names, out_names, out_avals, zero_shapes = _get_runner()
    concat_in = [np.asarray(global_map[name]) for name in in_names]
    concat_zeros = [
        np.zeros((NCORES * s[0], *s[1:]), dt) for s, dt in zero_shapes
    ]
    outs = sharded(*concat_in, *concat_zeros)
    outs = [np.asarray(o) for o in outs]
    return [
        {name: outs[i].reshape(NCORES, *out_avals[i].shape)[c]
         for i, name in enumerate(out_names)}
        for c in range(NCORES)
    ]


def kernel(traversal_lists, adj_matrices, ent_attn, spo_attn,
           ctx_idx_adjusted, roi_cls, roi_mask, weight_on_children):
    global_map = _prep_all(
        traversal_lists, adj_matrices, ent_attn, spo_attn,
        ctx_idx_adjusted, roi_cls, roi_mask, weight_on_children)
    res = _dispatch(global_map)
    out = np.empty((BS, NSEQ, NB), dtype=np.float32)
    for k in range(NCORES):
        out[k * BPC:(k + 1) * BPC] = (
            res[k]["ea_out"].astype(np.float32).reshape(BPC, NSEQ, NB))
    return out
